# Initial kernel scaffold
#
"""Cluster-local attention Trainium2 kernel.

Reference semantics (see problem):
    order = argsort(cluster_label, stable); xs = x[:, order]
    qkv = xs @ W_qkv + b_qkv ; q,k,v split, 8 heads x 64
    per (head, window of 256 sorted tokens): softmax(q k^T / 8) @ v
    h = attn_out @ W_out + b_out + xs            (returned in sorted order)

Sharding: 64 windows of 256 tokens -> 8 windows (2048 tokens) per core.
Fully embarrassingly parallel (weights replicated, no collectives).

Per-core device program (all matmuls bf16, accumulation fp32 in PSUM):
  1. load x fp32 (residual) and host-pretransposed X^T bf16 [512, 2048]
  2. Q^T,K^T via W-stationary qkv^T-form matmuls, kept [dim, tok];
     V via X^T-stationary token-major matmuls, copied into per-head
     80-col slots with a ones column at +64 for the softmax row-sum
  3. per (window, head):  s^T[k, q] = (K^T)^T Q^T ; exp on ACT (scale=1/8);
     PV matmul with the ones column producing [q, 64 out | rowsum];
     normalize via reciprocal + tensor_scalar_mul -> ao [tok, 512] bf16
  4. DMA-transpose ao -> ao^T; h = ao^T-form matmul + residual x (fp32)
"""

import sys

if "/opt/trn_rl_repo" not in sys.path:
    sys.path.insert(0, "/opt/trn_rl_repo")

import numpy as np
import ml_dtypes

B = 1
L = 16384
HID = 512
NH = 8
D = 64
WIN = 256
N_CORES = 8
T = L // N_CORES            # 2048 tokens per core
TT = T // 128               # 16 token tiles per core
NWIN = T // WIN             # 8 windows per core
VSLOT = 80                  # per-head column slot in V65 (64 v + 1 ones + pad)

_PROGRAM_CACHE = {}


def _build_program(has_bq: bool, has_bo: bool):
    import concourse.bacc as bacc
    import concourse.tile as tile
    import concourse.mybir as mybir

    fp32 = mybir.dt.float32
    bf16 = mybir.dt.bfloat16
    AF = mybir.ActivationFunctionType

    nc = bacc.Bacc("TRN2", target_bir_lowering=False, debug=False,
                   enable_asserts=False, num_devices=N_CORES)

    x_d = nc.dram_tensor("x", [T, HID], fp32, kind="ExternalInput").ap()
    xt_d = nc.dram_tensor("xt", [HID, T], bf16, kind="ExternalInput").ap()
    wqkv_d = nc.dram_tensor("wqkv", [HID, 3 * HID], bf16, kind="ExternalInput").ap()
    wout_d = nc.dram_tensor("wout", [HID, HID], bf16, kind="ExternalInput").ap()
    if has_bq:
        bqkv_d = nc.dram_tensor("bqkv", [3 * HID], fp32, kind="ExternalInput").ap()
    if has_bo:
        bout_d = nc.dram_tensor("bout", [HID], fp32, kind="ExternalInput").ap()
    h_d = nc.dram_tensor("h", [T, HID], fp32, kind="ExternalOutput").ap()

    import os
    debug = bool(os.environ.get("CLA_DEBUG"))
    stages = int(os.environ.get("CLA_STAGES", "4"))
    if debug:
        qkt_d = nc.dram_tensor("dbg_qkt", [128, 8 * T], bf16,
                               kind="ExternalOutput").ap()
        v65_d = nc.dram_tensor("dbg_v65", [128, TT * NH * 65], bf16,
                               kind="ExternalOutput").ap()
        ao_d = nc.dram_tensor("dbg_ao", [128, TT * HID], bf16,
                              kind="ExternalOutput").ap()
        ex_d = nc.dram_tensor("dbg_ex", [128, 2 * WIN], bf16,
                              kind="ExternalOutput").ap()

    from contextlib import ExitStack

    with tile.TileContext(nc) as tc, ExitStack() as ctx:
        consts = ctx.enter_context(tc.tile_pool(name="consts", bufs=1))
        vt_p = ctx.enter_context(tc.tile_pool(name="vt", bufs=4))
        exp_p = ctx.enter_context(tc.tile_pool(name="expp", bufs=4))
        h_p = ctx.enter_context(tc.tile_pool(name="hp", bufs=4))
        rc_p = ctx.enter_context(tc.tile_pool(name="rcp", bufs=8))
        import os
        _aot_pe = os.environ.get("CLA_AOT", "pe") == "pe"
        proj_ps = ctx.enter_context(tc.tile_pool(
            name="proj_ps", bufs=int(os.environ.get("CLA_PJB", "2")), space="PSUM"))
        st_ps = ctx.enter_context(tc.tile_pool(
            name="st_ps", bufs=int(os.environ.get("CLA_STB", "2")), space="PSUM"))
        pv_ps = ctx.enter_context(tc.tile_pool(
            name="pv_ps",
            bufs=int(os.environ.get("CLA_PVB", "2" if _aot_pe else "4")),
            space="PSUM"))
        if _aot_pe:
            tp_ps = ctx.enter_context(tc.tile_pool(
                name="tp_ps", bufs=2, space="PSUM"))


        # ---- persistent SBUF tensors -------------------------------------
        xf = consts.tile([128, TT * HID], fp32)           # x fp32, token tiles
        wqkv = consts.tile([128, 4 * 3 * HID], bf16)      # 4 hidden chunks
        wout = consts.tile([128, 4 * HID], bf16)
        xt = consts.tile([128, 4 * T], bf16)              # X^T, 4 hidden chunks
        qkt = consts.tile([128, 8 * T], bf16)             # Q^T|K^T, 8 dim chunks
        v65 = consts.tile([128, TT * NH * VSLOT], bf16)   # token-major V+ones
        ao = consts.tile([128, TT * HID], bf16)           # attn out token-major
        aot = consts.tile([128, 4 * T], bf16)             # ao^T, 4 chunks

        # CLA_REPEAT > 1 re-emits the whole body for dispatch-overhead-free
        # timing (wall(R) - wall(1) = (R-1) * kernel_time).
        repeat = int(os.environ.get("CLA_REPEAT", "1"))
        for _rep in range(repeat):
            _emit_body(nc, tc, mybir, locals())

    nc.compile()
    return nc


def _emit_body(nc, tc, mybir, env):
    import os
    fp32 = mybir.dt.float32
    bf16 = mybir.dt.bfloat16
    AF = mybir.ActivationFunctionType
    (has_bq, has_bo, debug, stages) = (env["has_bq"], env["has_bo"],
                                       env["debug"], env["stages"])
    (consts, vt_p, exp_p, h_p, rc_p) = (env["consts"], env["vt_p"],
                                        env["exp_p"], env["h_p"], env["rc_p"])
    (proj_ps, st_ps, pv_ps) = (env["proj_ps"], env["st_ps"], env["pv_ps"])
    tp_ps = env.get("tp_ps")
    (xf, wqkv, wout, xt, qkt, v65, ao, aot) = (
        env["xf"], env["wqkv"], env["wout"], env["xt"], env["qkt"],
        env["v65"], env["ao"], env["aot"])
    (x_d, xt_d, wqkv_d, wout_d, h_d) = (env["x_d"], env["xt_d"],
                                        env["wqkv_d"], env["wout_d"],
                                        env["h_d"])
    if has_bq:
        bqkv_d = env["bqkv_d"]
    if has_bo:
        bout_d = env["bout_d"]
    if debug:
        (qkt_d, v65_d, ao_d, ex_d) = (env["qkt_d"], env["v65_d"],
                                      env["ao_d"], env["ex_d"])

    if True:
        # ---- loads -------------------------------------------------------
        nc.sync.dma_start(
            out=xf.rearrange("p (t d) -> p t d", d=HID),
            in_=x_d.rearrange("(t p) d -> p t d", p=128))
        for c in range(4):
            nc.sync.dma_start(
                out=xt[:, c * T:(c + 1) * T],
                in_=xt_d[c * 128:(c + 1) * 128, :])
            nc.sync.dma_start(
                out=wqkv[:, c * 3 * HID:(c + 1) * 3 * HID],
                in_=wqkv_d[c * 128:(c + 1) * 128, :])
            nc.sync.dma_start(
                out=wout[:, c * HID:(c + 1) * HID],
                in_=wout_d[c * 128:(c + 1) * 128, :])

        if has_bq or has_bo:
            ones_row = consts.tile([1, 128], bf16)
            nc.vector.memset(ones_row, 1.0)
        if has_bq:
            bq_cols = consts.tile([128, 12], fp32)
            nc.sync.dma_start(out=bq_cols,
                              in_=bqkv_d.rearrange("(m p) -> p m", p=128))
            bqv_f = consts.tile([1, HID], fp32)
            nc.sync.dma_start(out=bqv_f,
                              in_=bqkv_d[2 * HID:3 * HID].rearrange(
                                  "(o d) -> o d", o=1))
            bqv_row = consts.tile([1, HID], bf16)
            nc.vector.tensor_copy(bqv_row, bqv_f)
        if has_bo:
            bout_f = consts.tile([1, HID], fp32)
            nc.sync.dma_start(out=bout_f,
                              in_=bout_d.rearrange("(o d) -> o d", o=1))
            bout_bf = consts.tile([1, HID], bf16)
            nc.vector.tensor_copy(bout_bf, bout_f)
            bps = proj_ps.tile([128, HID], fp32, tag="ps")
            nc.tensor.matmul(bps, ones_row, bout_bf, start=True, stop=True)
            bbc = consts.tile([128, HID], fp32)
            nc.vector.tensor_copy(bbc, bps)

        # ones columns of V65 (col 64 of every 80-col head slot)
        nc.vector.memset(
            v65.rearrange("p (s c) -> p s c", c=VSLOT)[:, :, 64:65], 1.0)

        # ---- Q^T / K^T projections (qkv^T form, W stationary) -------------
        copy_flip = 0
        for m in range(8):
            for n in range(4):  # token chunks of 512
                ps = proj_ps.tile([128, HID], fp32, tag="ps")
                for kk in range(4):
                    nc.tensor.matmul(
                        ps,
                        wqkv[:, kk * 3 * HID + m * 128: kk * 3 * HID + (m + 1) * 128],
                        xt[:, kk * T + n * 512: kk * T + (n + 1) * 512],
                        start=(kk == 0), stop=(kk == 3))
                dst = qkt[:, m * T + n * 512: m * T + (n + 1) * 512]
                if copy_flip == 0:
                    if has_bq:
                        nc.vector.tensor_scalar_add(dst, ps, bq_cols[:, m:m + 1])
                    else:
                        nc.vector.tensor_copy(dst, ps)
                else:
                    if has_bq:
                        nc.scalar.activation(dst, ps, AF.Identity,
                                             bias=bq_cols[:, m:m + 1])
                    else:
                        nc.scalar.activation(dst, ps, AF.Copy)
                copy_flip ^= 1

        # ---- V projection, token-major (X^T stationary) -------------------
        for t in range(TT):
            ps = proj_ps.tile([128, HID], fp32, tag="ps")
            if has_bq:
                nc.tensor.matmul(ps, ones_row, bqv_row,
                                 start=True, stop=False)
            for kk in range(4):
                nc.tensor.matmul(
                    ps,
                    xt[:, kk * T + t * 128: kk * T + (t + 1) * 128],
                    wqkv[:, kk * 3 * HID + 2 * HID: kk * 3 * HID + 3 * HID],
                    start=(kk == 0 and not has_bq), stop=(kk == 3))
            vt = vt_p.tile([128, HID], bf16)
            nc.vector.tensor_copy(vt, ps)
            # one strided copy scatters all 8 head slices into their slots
            nc.gpsimd.tensor_copy(
                v65[:, t * NH * VSLOT: (t + 1) * NH * VSLOT]
                .rearrange("p (h c) -> p h c", c=VSLOT)[:, :, 0:64],
                vt.rearrange("p (h c) -> p h c", c=64))

        # ---- attention ----------------------------------------------------
        # heads_per_exp (1 or 2) trades ACT overhead against PSUM pressure
        hpe = int(os.environ.get("CLA_HPE", "1"))
        for w in range(NWIN if stages >= 2 else 0):
            for hp in range(NH // hpe):
                st = st_ps.tile([128, hpe * 2 * WIN], fp32)
                for hi in range(hpe):
                    hh = hpe * hp + hi
                    mq = hh // 2
                    mk = 4 + hh // 2
                    prow = (hh % 2) * 64
                    for kc in range(2):
                        nc.tensor.matmul(
                            st[:, hi * 2 * WIN + kc * WIN:
                               hi * 2 * WIN + (kc + 1) * WIN],
                            qkt[prow:prow + 64,
                                mk * T + w * WIN + kc * 128:
                                mk * T + w * WIN + (kc + 1) * 128],
                            qkt[prow:prow + 64,
                                mq * T + w * WIN: mq * T + (w + 1) * WIN],
                            start=True, stop=True)
                ex = exp_p.tile([128, hpe * 2 * WIN], bf16)
                nc.scalar.activation(ex, st, AF.Exp, scale=1.0 / np.sqrt(D))
                if debug and w == 0 and hp == 0:
                    nc.sync.dma_start(out=ex_d, in_=ex[:, 0:2 * WIN])
                for hi in range(hpe):
                    hh = hpe * hp + hi
                    # both q-chunks of one head share a 1-bank psum tile:
                    # qc0 at cols 0:65, qc1 at 65:130 -> one reciprocal op
                    pv = pv_ps.tile([128, 130], fp32)
                    for qc in range(2):
                        for kc in range(2):
                            nc.tensor.matmul(
                                pv[:, qc * 65: qc * 65 + 65],
                                ex[:, hi * 2 * WIN + kc * WIN + qc * 128:
                                   hi * 2 * WIN + kc * WIN + (qc + 1) * 128],
                                v65[:, (2 * w + kc) * NH * VSLOT + hh * VSLOT:
                                    (2 * w + kc) * NH * VSLOT + hh * VSLOT + 65],
                                start=(kc == 0), stop=(kc == 1))
                    rc = rc_p.tile([128, 2], fp32)
                    nc.vector.reciprocal(
                        rc, pv.rearrange("p (q c) -> p q c", c=65)[:, :, 64])
                    for qc in range(2):
                        t = 2 * w + qc
                        nc.vector.tensor_scalar_mul(
                            ao[:, t * HID + hh * 64: t * HID + (hh + 1) * 64],
                            pv[:, qc * 65: qc * 65 + 64],
                            rc[:, qc:qc + 1])

        if debug:
            nc.sync.dma_start(out=qkt_d, in_=qkt)
            nc.sync.dma_start(
                out=v65_d.rearrange("p (s c) -> p s c", c=65),
                in_=v65.rearrange("p (s c) -> p s c", c=VSLOT)[:, :, 0:65])
            nc.sync.dma_start(out=ao_d, in_=ao)

        # ---- ao^T ---------------------------------------------------------
        aot_mode = os.environ.get("CLA_AOT", "pe")
        if aot_mode == "pe" and stages >= 3:
            from concourse.masks import make_identity
            ident = consts.tile([128, 128], bf16)
            make_identity(nc, ident)
            tp_flip = 0
            # two PE transposes (chunks c, c+1 of one token tile) share a
            # psum tile; one copy moves both to SBUF.  aot chunks c and c+1
            # sit T apart, so the copy uses a strided 3D dest AP.
            for t in range(TT):
                for c2 in range(2):
                    tp = tp_ps.tile([128, 256], bf16)
                    for j in range(2):
                        c = 2 * c2 + j
                        nc.tensor.transpose(
                            tp[:, j * 128:(j + 1) * 128],
                            ao[:, t * HID + c * 128: t * HID + (c + 1) * 128],
                            ident)
                    dst = aot.rearrange("p (c x) -> p c x", x=T)[
                        :, 2 * c2: 2 * c2 + 2, t * 128:(t + 1) * 128]
                    if tp_flip == 0:
                        nc.vector.tensor_copy(
                            dst, tp.rearrange("p (j x) -> p j x", x=128))
                    else:
                        nc.scalar.activation(
                            dst, tp.rearrange("p (j x) -> p j x", x=128),
                            AF.Copy)
                    tp_flip ^= 1
        else:
            for t in range(TT if stages >= 3 else 0):
                for c in range(4):
                    nc.sync.dma_start(
                        out=aot[:, c * T + t * 128: c * T + (t + 1) * 128],
                        in_=ao[:, t * HID + c * 128: t * HID + (c + 1) * 128],
                        transpose=True)

        # ---- output projection + residual ---------------------------------
        for t in range(TT if stages >= 4 else 0):
            ps = proj_ps.tile([128, HID], fp32, tag="ps")
            for kk in range(4):
                nc.tensor.matmul(
                    ps,
                    aot[:, kk * T + t * 128: kk * T + (t + 1) * 128],
                    wout[:, kk * HID:(kk + 1) * HID],
                    start=(kk == 0), stop=(kk == 3))
            ht = h_p.tile([128, HID], fp32)
            nc.vector.tensor_add(ht, ps, xf[:, t * HID:(t + 1) * HID])
            if has_bo:
                nc.vector.tensor_add(ht, ht, bbc)
            nc.gpsimd.dma_start(out=h_d[t * 128:(t + 1) * 128, :], in_=ht)

    nc.compile()
    return nc


def _get_program(has_bq: bool, has_bo: bool):
    key = (has_bq, has_bo)
    if key not in _PROGRAM_CACHE:
        _PROGRAM_CACHE[key] = _build_program(has_bq, has_bo)
    return _PROGRAM_CACHE[key]


def make_in_maps(x, cluster_label, W_qkv, b_qkv, W_out, b_out):
    """Host-side prep: sort tokens, shard, cast weights. Returns
    (in_maps, has_bq, has_bo)."""
    x = np.asarray(x, dtype=np.float32).reshape(L, HID)
    labels = np.asarray(cluster_label)
    order = np.argsort(labels, kind="stable")
    if not np.array_equal(order, np.arange(L)):
        xs = np.ascontiguousarray(x[order])
    else:
        xs = np.ascontiguousarray(x)
    wqkv_bf = np.asarray(W_qkv, dtype=np.float32).astype(ml_dtypes.bfloat16)
    wout_bf = np.asarray(W_out, dtype=np.float32).astype(ml_dtypes.bfloat16)
    bq = np.asarray(b_qkv, dtype=np.float32).reshape(3 * HID)
    bo = np.asarray(b_out, dtype=np.float32).reshape(HID)
    has_bq = bool(np.any(bq != 0))
    has_bo = bool(np.any(bo != 0))

    xs_bf = xs.astype(ml_dtypes.bfloat16)
    in_maps = []
    for c in range(N_CORES):
        m = {
            "x": xs[c * T:(c + 1) * T],
            "xt": np.ascontiguousarray(xs_bf[c * T:(c + 1) * T].T),
            "wqkv": wqkv_bf,
            "wout": wout_bf,
        }
        if has_bq:
            m["bqkv"] = bq
        if has_bo:
            m["bout"] = bo
        in_maps.append(m)
    return in_maps, has_bq, has_bo


def kernel(x, cluster_label, W_qkv, b_qkv, W_out, b_out):
    from concourse.bass_utils import run_bass_kernel_spmd

    in_maps, has_bq, has_bo = make_in_maps(
        x, cluster_label, W_qkv, b_qkv, W_out, b_out)
    nc = _get_program(has_bq, has_bo)
    res = run_bass_kernel_spmd(nc, in_maps, list(range(N_CORES)), trace=False)
    h = np.concatenate([res.results[c]["h"] for c in range(N_CORES)], axis=0)
    return h.reshape(B, L, HID).astype(np.float32)



# revision 21
# speedup vs baseline: 11.0287x; 11.0287x over previous
"""Cluster-local attention Trainium2 kernel (v2: fp8 DoubleRow).

Reference semantics:
    order = argsort(cluster_label, stable); xs = x[:, order]
    qkv = xs @ W_qkv + b_qkv ; q,k,v split, 8 heads x 64
    per (head, window of 256 sorted tokens): softmax(q k^T / 8) @ v
    h = attn_out @ W_out + b_out + xs            (returned in sorted order)

Sharding: 64 windows of 256 tokens -> 8 windows (2048 tokens) per core,
weights replicated, no collectives.

Numerics: the output is dominated by the fp32 residual (attention path is
~1.3% of |h|), so the attention path runs in fp8/bf16:
  - W_qkv, W_out host-scaled x32 and cast to fp8e4; x^T cast to fp8e4.
  - QKV / out projections use fp8 DoubleRow matmuls (2 k-tiles per pass).
  - scores S^T[k,q] in bf16 (64-deep contraction, no DR possible);
    exp on ACT -> fp8 probs; PV is one DoubleRow matmul per (head, qc)
    with a ones column at slot 64 giving the softmax row-sum.
  - normalize on DVE/Pool via reciprocal + broadcast tensor_tensor -> ao bf16
  - ao -> ao^T via DMA transpose (xbar); convert to fp8; out-projection in
    h^T form (fp8 DR); epilogue h^T = psum/1024 + x^T (fp32) -> DRAM.
Host side: pre-transpose x, cast weights; final h^T -> h transpose.
"""

import sys

if "/opt/trn_rl_repo" not in sys.path:
    sys.path.insert(0, "/opt/trn_rl_repo")

import os
import numpy as np
import ml_dtypes

B = 1
L = 16384
HID = 512
NH = 8
D = 64
WIN = 256
N_CORES = 8
T = L // N_CORES            # 2048 tokens per core
TT = T // 128               # 16 token tiles per core
NWIN = T // WIN             # 8 windows per core
VS = 72                     # per-head V slot: 64 v + 1 ones + 7 pad (alignment for fp8 DR)
WSCALE = 32.0               # host scale on W_qkv / W_out for fp8 range

_PROGRAM_CACHE = {}


def _build_program(has_bq: bool, has_bo: bool):
    import concourse.bacc as bacc
    import concourse.tile as tile
    import concourse.mybir as mybir

    fp32 = mybir.dt.float32
    bf16 = mybir.dt.bfloat16
    fp8 = mybir.dt.float8e4
    DR = mybir.MatmulPerfMode.DoubleRow

    nc = bacc.Bacc("TRN2", target_bir_lowering=False, debug=False,
                   enable_asserts=False, num_devices=N_CORES)

    xtf_d = nc.dram_tensor("xtf", [HID, T], fp32, kind="ExternalInput").ap()
    xt8_d = nc.dram_tensor("xt8", [HID, T], fp8, kind="ExternalInput").ap()
    w8_d = nc.dram_tensor("w8", [HID, 3 * HID], fp8, kind="ExternalInput").ap()
    wo8_d = nc.dram_tensor("wo8", [HID, HID], fp8, kind="ExternalInput").ap()
    if has_bq:
        bq_d = nc.dram_tensor("bq", [3 * HID], fp32, kind="ExternalInput").ap()
    if has_bo:
        bo_d = nc.dram_tensor("bo", [HID], fp32, kind="ExternalInput").ap()
    ht_d = nc.dram_tensor("ht", [HID, T], fp32, kind="ExternalOutput").ap()

    from contextlib import ExitStack

    with tile.TileContext(nc) as tc, ExitStack() as ctx:
        consts = ctx.enter_context(tc.tile_pool(name="consts", bufs=1))
        ex_p = ctx.enter_context(tc.tile_pool(name="exp", bufs=3))
        rc_p = ctx.enter_context(tc.tile_pool(name="rcp", bufs=4))
        proj_ps = ctx.enter_context(tc.tile_pool(
            name="proj_ps", bufs=2, space="PSUM"))
        st_ps = ctx.enter_context(tc.tile_pool(
            name="st_ps", bufs=2, space="PSUM"))
        pv_ps = ctx.enter_context(tc.tile_pool(
            name="pv_ps", bufs=2, space="PSUM"))

        # ---- persistent SBUF tensors -------------------------------------
        env = dict(
            xtf=consts.tile([128, 4 * T], fp32, name="xtf"),
            xt8=consts.tile([128, 4 * T], fp8, name="xt8"),
            w8=consts.tile([128, 4 * 3 * HID], fp8, name="w8"),
            wo8=consts.tile([128, 4 * HID], fp8, name="wo8"),
            qkt=consts.tile([128, 8 * T], bf16, name="qkt"),
            v65=consts.tile([128, TT * NH * VS], fp8, name="v65"),
            ao=consts.tile([128, TT * HID], bf16, name="ao"),
            aot=consts.tile([128, 4 * T], bf16, name="aot"),
            ao8=consts.tile([128, 4 * T], fp8, name="ao8"),
            hT=consts.tile([128, 4 * T], fp32, name="hT"),
            consts=consts, ex_p=ex_p, rc_p=rc_p,
            proj_ps=proj_ps, st_ps=st_ps, pv_ps=pv_ps,
            has_bq=has_bq, has_bo=has_bo,
            xtf_d=xtf_d, xt8_d=xt8_d, w8_d=w8_d, wo8_d=wo8_d, ht_d=ht_d,
        )
        if has_bq:
            env["bq_d"] = bq_d
        if has_bo:
            env["bo_d"] = bo_d

        repeat = int(os.environ.get("CLA_REPEAT", "1"))
        for _rep in range(repeat):
            _emit_body(nc, mybir, env)

    nc.compile()
    return nc


def _emit_body(nc, mybir, env):
    fp32 = mybir.dt.float32
    bf16 = mybir.dt.bfloat16
    fp8 = mybir.dt.float8e4
    AF = mybir.ActivationFunctionType
    DR = mybir.MatmulPerfMode.DoubleRow
    MUL = mybir.AluOpType.mult

    has_bq, has_bo = env["has_bq"], env["has_bo"]
    xtf, xt8, w8, wo8 = env["xtf"], env["xt8"], env["w8"], env["wo8"]
    qkt, v65, ao, aot, ao8, hT = (env["qkt"], env["v65"], env["ao"],
                                  env["aot"], env["ao8"], env["hT"])
    consts, ex_p, rc_p = env["consts"], env["ex_p"], env["rc_p"]
    proj_ps, st_ps, pv_ps = env["proj_ps"], env["st_ps"], env["pv_ps"]
    xtf_d, xt8_d, w8_d, wo8_d, ht_d = (env["xtf_d"], env["xt8_d"],
                                       env["w8_d"], env["wo8_d"],
                                       env["ht_d"])

    W3 = 3 * HID  # 1536

    # ---- loads: critical-path tensors on SP; residual x^T via SWDGE ------
    # interleaved so the first DR pair (chunks 0,1) is ready earliest
    for c in (0, 1):
        nc.sync.dma_start(out=xt8[:, c * T:(c + 1) * T],
                          in_=xt8_d[c * 128:(c + 1) * 128, :])
    for c in (0, 1):
        nc.scalar.dma_start(out=w8[:, c * W3:(c + 1) * W3],
                            in_=w8_d[c * 128:(c + 1) * 128, :])
    for c in (2, 3):
        nc.sync.dma_start(out=xt8[:, c * T:(c + 1) * T],
                          in_=xt8_d[c * 128:(c + 1) * 128, :])
    for c in (2, 3):
        nc.scalar.dma_start(out=w8[:, c * W3:(c + 1) * W3],
                            in_=w8_d[c * 128:(c + 1) * 128, :])
    # xtf per (m, n-chunk) slices: rep i+1's slice load only waits on the
    # matching epilogue read of rep i, keeping CLA_REPEAT reps pipelined
    for c in range(4):
        for nn in range(4):
            nc.gpsimd.dma_start(
                out=xtf[:, c * T + nn * 512: c * T + (nn + 1) * 512],
                in_=xtf_d[c * 128:(c + 1) * 128, nn * 512:(nn + 1) * 512])
    for c in range(4):
        nc.sync.dma_start(out=wo8[:, c * HID:(c + 1) * HID],
                          in_=wo8_d[c * 128:(c + 1) * 128, :])

    if has_bq:
        bq_cols = consts.tile([128, 12], fp32)
        nc.sync.dma_start(out=bq_cols,
                          in_=env["bq_d"].rearrange("(m p) -> p m", p=128))
        ones_row = consts.tile([1, 128], fp8)
        nc.vector.memset(ones_row, 1.0)
        bqv_f = consts.tile([1, HID], fp32)
        nc.sync.dma_start(out=bqv_f,
                          in_=env["bq_d"][2 * HID:3 * HID].rearrange(
                              "(o d) -> o d", o=1))
        bqv_row = consts.tile([1, HID], fp8)
        nc.vector.tensor_copy(bqv_row, bqv_f)
    if has_bo:
        bo_cols = consts.tile([128, 4], fp32)
        nc.sync.dma_start(out=bo_cols,
                          in_=env["bo_d"].rearrange("(m p) -> p m", p=128))

    # ones column at 64, zero pad at 65:72 of every VS-col head slot
    nc.any.memset(
        v65.rearrange("p (s c) -> p s c", c=VS)[:, :, 64:65], 1.0)
    nc.any.memset(
        v65.rearrange("p (s c) -> p s c", c=VS)[:, :, 65:VS], 0.0)

    conv_pool = os.environ.get("CLA_CONV", "pool") == "pool"
    stage = int(os.environ.get("CLA_STAGE", "7"))

    def out_proj_half(n, half):
        if stage < 7:
            return
        for m in range(4):
            psf = proj_ps.tile([128, HID], fp32, tag="ps")
            ps = psf[:, 0:256]
            lo = n * 512 + half * 256
            for kp in range(2):
                nc.tensor.matmul(
                    ps,
                    wo8.rearrange("p (kk x) -> p kk x", x=HID)[
                        :, 2 * kp:2 * kp + 2, m * 128:(m + 1) * 128],
                    ao8.rearrange("p (kk t) -> p kk t", t=T)[
                        :, 2 * kp:2 * kp + 2, lo:lo + 256],
                    start=(kp == 0), stop=(kp == 1), perf_mode=DR)
            dst = hT[:, m * T + lo: m * T + lo + 256]
            nc.vector.scalar_tensor_tensor(
                out=dst, in0=ps, scalar=1.0 / (WSCALE * WSCALE),
                op0=MUL,
                in1=xtf[:, m * T + lo: m * T + lo + 256],
                op1=mybir.AluOpType.add)
            if has_bo:
                nc.vector.tensor_scalar_add(dst, dst, bo_cols[:, m:m + 1])
            if half == 1:
                nc.gpsimd.dma_start(
                    out=ht_d[m * 128:(m + 1) * 128, n * 512:(n + 1) * 512],
                    in_=hT[:, m * T + n * 512: m * T + (n + 1) * 512])

    def out_proj(n):
        if stage < 7:
            return
        # ---- out projection (h^T form) + residual epilogue, chunk n ------
        for m in range(4):
            ps = proj_ps.tile([128, HID], fp32, tag="ps")
            for kp in range(2):
                nc.tensor.matmul(
                    ps,
                    wo8.rearrange("p (kk x) -> p kk x", x=HID)[
                        :, 2 * kp:2 * kp + 2, m * 128:(m + 1) * 128],
                    ao8.rearrange("p (kk t) -> p kk t", t=T)[
                        :, 2 * kp:2 * kp + 2, n * 512:(n + 1) * 512],
                    start=(kp == 0), stop=(kp == 1), perf_mode=DR)
            dst = hT[:, m * T + n * 512: m * T + (n + 1) * 512]
            nc.vector.scalar_tensor_tensor(
                out=dst, in0=ps, scalar=1.0 / (WSCALE * WSCALE),
                op0=MUL,
                in1=xtf[:, m * T + n * 512: m * T + (n + 1) * 512],
                op1=mybir.AluOpType.add)
            if has_bo:
                nc.vector.tensor_scalar_add(dst, dst, bo_cols[:, m:m + 1])
            nc.gpsimd.dma_start(
                out=ht_d[m * 128:(m + 1) * 128, n * 512:(n + 1) * 512],
                in_=dst)

    # token-chunk-major emission so window w's inputs are produced early;
    # out_proj runs one chunk behind so PE never waits on transpose/convert
    cflip = 0
    carry = None
    for n in range(4):
        # ---- Q^T / K^T projection chunks (DoubleRow, W stationary) -------
        for m in range(8 if stage >= 2 else 0):
            ps = proj_ps.tile([128, HID], fp32, tag="ps")
            for kp in range(2):
                nc.tensor.matmul(
                    ps,
                    w8.rearrange("p (kk w) -> p kk w", w=W3)[
                        :, 2 * kp:2 * kp + 2, m * 128:(m + 1) * 128],
                    xt8.rearrange("p (kk t) -> p kk t", t=T)[
                        :, 2 * kp:2 * kp + 2, n * 512:(n + 1) * 512],
                    start=(kp == 0), stop=(kp == 1), perf_mode=DR)
            dst = qkt[:, m * T + n * 512: m * T + (n + 1) * 512]
            if cflip in (0, 2):
                if has_bq:
                    nc.vector.tensor_scalar_add(dst, ps, bq_cols[:, m:m + 1])
                else:
                    nc.vector.tensor_copy(dst, ps)
            else:
                if has_bq:
                    nc.scalar.activation(dst, ps, AF.Identity,
                                         bias=bq_cols[:, m:m + 1])
                else:
                    nc.scalar.activation(dst, ps, AF.Copy)
            cflip = (cflip + 1) % 5

        # ---- V projection token tiles 4n..4n+4 (DoubleRow, X stationary) -
        for t in range(4 * n, (4 * n + 4) if stage >= 3 else (4 * n)):
            ps = proj_ps.tile([128, HID], fp32, tag="ps")
            if has_bq:
                nc.tensor.matmul(ps, ones_row, bqv_row,
                                 start=True, stop=False)
            for kp in range(2):
                nc.tensor.matmul(
                    ps,
                    xt8.rearrange("p (kk t) -> p kk t", t=T)[
                        :, 2 * kp:2 * kp + 2, t * 128:(t + 1) * 128],
                    w8.rearrange("p (kk w) -> p kk w", w=W3)[
                        :, 2 * kp:2 * kp + 2, 2 * HID:3 * HID],
                    start=(kp == 0 and not has_bq), stop=(kp == 1),
                    perf_mode=DR)
            # strided copy scatters all 8 head slices into their 65-col slots
            nc.vector.tensor_copy(
                v65[:, t * NH * VS: (t + 1) * NH * VS]
                .rearrange("p (h c) -> p h c", c=VS)[:, :, 0:64],
                ps.rearrange("p (h c) -> p h c", c=64))

        # ---- attention for windows 2n, 2n+1 ------------------------------
        # (out_proj(n-1) slotted between the windows: PE fills exp gaps)
        att = int(os.environ.get("CLA_ATT", "5"))

        def scores_block(w, hp):
            st = st_ps.tile([128, 2 * 2 * WIN], fp32)
            for hi in range(2):
                hh = 2 * hp + hi
                mq = hh // 2
                mk = 4 + hh // 2
                prow = (hh % 2) * 64
                for kc in range(2):
                    nc.tensor.matmul(
                        st[:, hi * 2 * WIN + kc * WIN:
                           hi * 2 * WIN + (kc + 1) * WIN],
                        qkt[prow:prow + 64,
                            mk * T + w * WIN + kc * 128:
                            mk * T + w * WIN + (kc + 1) * 128],
                        qkt[prow:prow + 64,
                            mq * T + w * WIN: mq * T + (w + 1) * WIN],
                        start=True, stop=True)
            if att < 2:
                return None
            ex = ex_p.tile([128, 2 * 2 * WIN], fp8)
            nc.scalar.activation(ex, st, AF.Exp,
                                 scale=1.0 / (np.sqrt(D) * WSCALE ** 2))
            return ex

        def pv_block(w, hp, ex):
            if att < 3 or ex is None:
                return
            pv = pv_ps.tile([128, 4 * VS], fp32)
            for hi in range(2):
                hh = 2 * hp + hi
                for qc in range(2):
                    nc.tensor.matmul(
                        pv[:, (2 * hi + qc) * VS:(2 * hi + qc + 1) * VS],
                        ex.rearrange("p (h kc q) -> p h kc q", h=2, kc=2)[
                            :, hi, :, qc * 128:(qc + 1) * 128],
                        v65.rearrange("p (t s) -> p t s", s=NH * VS)[
                            :, 2 * w:2 * w + 2, hh * VS:(hh + 1) * VS],
                        start=True, stop=True, perf_mode=DR)
            if att < 4:
                return
            rc = rc_p.tile([128, 4], fp32)
            nc.vector.reciprocal(
                rc, pv.rearrange("p (s c) -> p s c", c=VS)[:, :, 64])
            if att < 5:
                return
            # ao[tok=2w+qc, head hh=2hp+hi] = pv[:, s=2hi+qc, 0:64]*rc[s]
            nc.vector.tensor_tensor(
                out=ao.rearrange("p (t x) -> p t x", x=HID)[
                    :, 2 * w:2 * w + 2,
                    2 * hp * D:(2 * hp + 2) * D].rearrange(
                        "p t (h d) -> p t h d", h=2),
                in0=pv.rearrange("p (h q c) -> p q h c", h=2, q=2)[
                    :, :, :, 0:64],
                in1=rc.rearrange("p (h q o) -> p q h o", h=2, o=1)
                    .broadcast_to([128, 2, 2, 64]),
                op=MUL)

        def post_window(w):
            # ao tiles 2w, 2w+1 complete: transpose + fp8-convert this
            # window's 256 token-columns; split out_proj(3) starts at w==6
            if stage >= 5:
                for t in (2 * w, 2 * w + 1):
                    nc.sync.dma_start(
                        out=aot.rearrange("p (c x) -> p c x", x=T)[
                            :, :, t * 128:(t + 1) * 128],
                        in_=ao[:, t * HID:(t + 1) * HID],
                        transpose=True)
            if stage >= 6:
                lo = w * 256
                conv_eng = nc.gpsimd if conv_pool else nc.vector
                conv_eng.tensor_copy(
                    ao8.rearrange("p (c x) -> p c x", x=T)[
                        :, :, lo:lo + 256],
                    aot.rearrange("p (c x) -> p c x", x=T)[
                        :, :, lo:lo + 256])
            if w == 6:
                out_proj_half(3, 0)

        def flush_pv(carry):
            pv_block(*carry)
            if carry[1] == 3:
                post_window(carry[0])

        def attn_window(w, carry):
            # emit scores(hp) then pv(hp-1): PV LDWEIGHTS hide under the
            # next head-pair's 256-col score streams
            for hp in range(4):
                ex = scores_block(w, hp)
                if carry is not None:
                    flush_pv(carry)
                carry = (w, hp, ex)
            return carry

        if stage >= 4:
            carry = attn_window(2 * n, carry)
        if n > 0:
            out_proj(n - 1)
        if stage >= 4:
            carry = attn_window(2 * n + 1, carry)


    if stage >= 4 and carry is not None:
        flush_pv(carry)
        carry = None
    out_proj_half(3, 1)


def _get_program(has_bq: bool, has_bo: bool):
    key = (has_bq, has_bo)
    if key not in _PROGRAM_CACHE:
        _PROGRAM_CACHE[key] = _build_program(has_bq, has_bo)
    return _PROGRAM_CACHE[key]


def make_in_maps(x, cluster_label, W_qkv, b_qkv, W_out, b_out):
    """Host-side prep: sort tokens, shard, transpose + cast. Returns
    (in_maps, has_bq, has_bo)."""
    x = np.asarray(x, dtype=np.float32).reshape(L, HID)
    labels = np.asarray(cluster_label)
    order = np.argsort(labels, kind="stable")
    if not np.array_equal(order, np.arange(L)):
        xs = np.ascontiguousarray(x[order])
    else:
        xs = x
    f8 = ml_dtypes.float8_e4m3
    w8 = (np.asarray(W_qkv, dtype=np.float32) * WSCALE).astype(f8)
    wo8 = (np.asarray(W_out, dtype=np.float32) * WSCALE).astype(f8)
    bq = np.asarray(b_qkv, dtype=np.float32).reshape(3 * HID)
    bo = np.asarray(b_out, dtype=np.float32).reshape(HID)
    has_bq = bool(np.any(bq != 0))
    has_bo = bool(np.any(bo != 0))

    in_maps = []
    for c in range(N_CORES):
        xtf = np.ascontiguousarray(xs[c * T:(c + 1) * T].T)
        m = {
            "xtf": xtf,
            "xt8": xtf.astype(f8),
            "w8": w8,
            "wo8": wo8,
        }
        if has_bq:
            m["bq"] = bq * WSCALE
        if has_bo:
            m["bo"] = bo
        in_maps.append(m)
    return in_maps, has_bq, has_bo


def kernel(x, cluster_label, W_qkv, b_qkv, W_out, b_out):
    from concourse.bass_utils import run_bass_kernel_spmd

    in_maps, has_bq, has_bo = make_in_maps(
        x, cluster_label, W_qkv, b_qkv, W_out, b_out)
    nc = _get_program(has_bq, has_bo)
    res = run_bass_kernel_spmd(nc, in_maps, list(range(N_CORES)), trace=False)
    h = np.concatenate(
        [np.asarray(res.results[c]["ht"]).T for c in range(N_CORES)], axis=0)
    return np.ascontiguousarray(h).reshape(B, L, HID).astype(np.float32)
